# revision 8
# baseline (speedup 1.0000x reference)
"""PPR diffusion P = theta1*(A + A^2 + A^3) + alpha*I on 8 NeuronCores.

Sharding: 1D row partition of P. Core c computes block-rows
[c*1024, (c+1)*1024) of A^2 and A^3 as transposed blocks:
  pass 1: M2T = (A_blk @ A).T = A.T @ A_blk.T   (stationary = A col panels)
  pass 2: M3T = (M2_blk @ A).T = A.T @ M2T      (same stationary stream)
M2T stays resident in SBUF in fp8; A is streamed once per pass as the
fp8 stationary operand (DoubleRow). The exact theta1*A term is added in
fp32 during assembly. Host does the O(E) scatter to densify A, the fp8
quantization (scale S1), and the final transpose + alpha*I.
"""

import numpy as np
import ml_dtypes

N = 8192
BLK = 1024
NCORES = 8
KP = 128          # partitions / tile edge
KT = N // KP      # 64 k-tiles
JT = N // KP      # 64 output column tiles
F = BLK           # free width of the resident block operands
HF = 512          # psum half width
S1 = 64.0         # fp8 quantization scale for A
ALPHA = 0.4
TH1 = ALPHA * (1.0 - ALPHA)
FP8_NP = ml_dtypes.float8_e4m3

_NC_CACHE = None


def _build_nc():
    import concourse.bacc as bacc
    import concourse.tile as tile
    from concourse import mybir

    f32 = mybir.dt.float32
    fp8 = mybir.dt.float8e4
    DR = mybir.MatmulPerfMode.DoubleRow
    MUL = mybir.AluOpType.mult
    ADD = mybir.AluOpType.add

    nc = bacc.Bacc(None, target_bir_lowering=False, debug=False)
    as_d = nc.dram_tensor("as_t", [JT, KP, KT, KP], fp8, kind="ExternalInput")
    abl_d = nc.dram_tensor("ablkt", [KP, KT, F], fp8, kind="ExternalInput")
    at_d = nc.dram_tensor("at_t", [JT, KP, F], f32, kind="ExternalInput")
    out_d = nc.dram_tensor("ppr_out", [JT, KP, F], f32, kind="ExternalOutput")

    with tile.TileContext(nc) as tc:
        with (
            tc.tile_pool(name="res", bufs=1) as res_pool,
            tc.tile_pool(name="stat", bufs=2) as stat_pool,
            tc.tile_pool(name="at", bufs=2) as at_pool,
            tc.tile_pool(name="t1", bufs=2) as t1_pool,
            tc.tile_pool(name="pt", bufs=2) as pt_pool,
            tc.tile_pool(name="psum", bufs=2, space="PSUM") as psum_pool,
        ):
            abl = res_pool.tile([KP, KT, F], fp8, name="abl")
            m2t = res_pool.tile([KP, KT, F], fp8, name="m2t")
            nc.sync.dma_start(abl[:], abl_d[:])

            for pass_i in range(2):
                mov = abl if pass_i == 0 else m2t
                for j in range(JT):
                    stat = stat_pool.tile([KP, KT, KP], fp8, name="stat")
                    nc.sync.dma_start(stat[:], as_d[j])
                    pa = psum_pool.tile([KP, HF], f32, name="pa")
                    pb = psum_pool.tile([KP, HF], f32, name="pb")
                    for kp_i in range(KT // 2):
                        st = kp_i == 0
                        sp = kp_i == KT // 2 - 1
                        lhsT = stat[:, 2 * kp_i:2 * kp_i + 2, :]
                        nc.tensor.matmul(
                            pa[:], lhsT, mov[:, 2 * kp_i:2 * kp_i + 2, 0:HF],
                            start=st, stop=sp, perf_mode=DR)
                        nc.tensor.matmul(
                            pb[:], lhsT, mov[:, 2 * kp_i:2 * kp_i + 2, HF:F],
                            start=st, stop=sp, perf_mode=DR)
                    if pass_i == 0:
                        nc.vector.tensor_copy(m2t[:, j, 0:HF], pa[:])
                        nc.vector.tensor_copy(m2t[:, j, HF:F], pb[:])
                    else:
                        at = at_pool.tile([KP, F], f32, name="at")
                        nc.sync.dma_start(at[:], at_d[j])
                        t1 = t1_pool.tile([KP, F], f32, name="t1")
                        pt = pt_pool.tile([KP, F], f32, name="pt")
                        nc.vector.scalar_tensor_tensor(
                            t1[:, 0:HF], pa[:], 1.0 / S1, m2t[:, j, 0:HF],
                            op0=MUL, op1=ADD)
                        nc.vector.scalar_tensor_tensor(
                            t1[:, HF:F], pb[:], 1.0 / S1, m2t[:, j, HF:F],
                            op0=MUL, op1=ADD)
                        nc.vector.scalar_tensor_tensor(
                            pt[:, 0:HF], t1[:, 0:HF], TH1 / (S1 * S1),
                            at[:, 0:HF], op0=MUL, op1=ADD)
                        nc.vector.scalar_tensor_tensor(
                            pt[:, HF:F], t1[:, HF:F], TH1 / (S1 * S1),
                            at[:, HF:F], op0=MUL, op1=ADD)
                        nc.sync.dma_start(out_d[j], pt[:])
    nc.compile()
    return nc


def _get_nc():
    global _NC_CACHE
    if _NC_CACHE is None:
        _NC_CACHE = _build_nc()
    return _NC_CACHE


def _prepare_in_maps(x, edge_attr, edge_index):
    row = np.asarray(edge_index[0], dtype=np.int64)
    col = np.asarray(edge_index[1], dtype=np.int64)
    ea = np.asarray(edge_attr, dtype=np.float32)

    deg = np.bincount(col, minlength=N).astype(np.float32)
    with np.errstate(divide="ignore"):
        dis = deg ** -0.5
    vals = dis[row] * ea * dis[col]

    flat = row * N + col
    A = (np.bincount(flat, weights=vals.astype(np.float64), minlength=N * N)
         .astype(np.float32).reshape(N, N))

    As8 = (A * S1).astype(FP8_NP)
    # [j, p, t, b] = As8[t*128+p, j*128+b]
    as_dma = np.ascontiguousarray(
        As8.reshape(KT, KP, JT, KP).transpose(2, 1, 0, 3))

    in_maps = []
    for c in range(NCORES):
        blk = slice(c * BLK, (c + 1) * BLK)
        ablT = np.ascontiguousarray(As8[blk].T)          # [N(k), F]
        abl = np.ascontiguousarray(
            ablT.reshape(KT, KP, F).transpose(1, 0, 2))  # [p, t, f]
        atT = np.ascontiguousarray(A[blk].T) * TH1       # [N(n'), F] f32
        at = atT.reshape(JT, KP, F)                      # [j, p, f]
        in_maps.append({"as_t": as_dma, "ablkt": abl, "at_t": at})
    return in_maps


def _assemble(results):
    P = np.empty((N, N), dtype=np.float32)
    for c in range(NCORES):
        o = np.asarray(results[c]["ppr_out"]).reshape(N, F)
        P[c * BLK:(c + 1) * BLK, :] = o.T
    P[np.arange(N), np.arange(N)] += ALPHA
    return P


def kernel(x, edge_attr, edge_index):
    from concourse.bass_utils import run_bass_kernel_spmd

    in_maps = _prepare_in_maps(x, edge_attr, edge_index)
    nc = _get_nc()
    res = run_bass_kernel_spmd(nc, in_maps, core_ids=list(range(NCORES)))
    return _assemble(res.results)
